# revision 25
# baseline (speedup 1.0000x reference)
"""Trainium2 Bass kernel for nn_AttentionLayer (self-attention over 64x64 images).

Computation (per batch image b):
    xf = x[b].reshape(C, N)                     # C=256, N=4096
    q = BN(Wq @ xf + bq)   -> [32, N]
    k = BN(Wk @ xf + bk)   -> [32, N]
    v = BN(Wv @ xf + bv)   -> [256, N]
    attn = softmax_j(q^T k) -> [N, N]
    out = v @ attn^T        -> [256, N]
    y = gamma * out + xf

Sharding: 8 cores = 4 batches x 2 query-row halves; no collectives.

Host-side algebra folds (all exact):
  - BN folded into weights/bias.
  - k bias drops entirely (adds a per-query constant to every logit row ->
    cancels in softmax); q bias kept (its logit term varies over keys).
  - v bias passes through softmax unchanged (rows sum to 1), so
    y = (gamma*vhat @ P)*recip + (x + gamma*bv); gamma folds into Wv and the
    residual bias rb = gamma*bv is added per-channel in the epilogue.
  - Each core's xbh has its own query-half columns FIRST (keys are
    order-invariant), so the query slice is a view of the key tensor and the
    bf16 x tile doubles as the residual.

Device (per core, all matmuls bf16 in / f32 psum):
  - Input DMAs in 3 column waves issued from three different queues
    (sync/gpsimd/scalar) so the ~0.7us per-issue cost is paid in parallel
    and the first projection matmul starts ~9.5us in.
  - q/k proj: 2 col-group replica matmuls -> [64, N*] (col-group pairs
    stream near-concurrently on the PE); q evicted via ACT Identity+bias,
    k via DVE copy.  Bands 64-127 are then filled by one SBUF->SBUF DMA
    per tensor, so S rounds can use 4 distinct PE row bands.
  - V^T[j, c] = x^T Wv^T computed directly in [j, c] layout.
  - Main loop over 4 i-blocks x 16 rounds of 2 key-chunks: S^T row-band
    matmul pair -> exp on ScalarE (no max subtraction; |S|<60 fits f32/bf16)
    -> P^T bf16 -> out psum accumulation + bf16 rowsum chains on DVE.
    Consecutive rounds use different PE row-band pairs (0/32 vs 64/96), so
    the adjacently emitted S pairs of rounds sr+2 and sr+3 stream 4-way
    concurrently.
  - Epilogue per i-block is split so no PE instruction ever waits on the
    rowsum->reciprocal chain: at the block's last round only the psum
    eviction happens; the rowsum matmuls + reciprocal + GpSimd
    partition_broadcast flush after the NEXT block's round-0 outs, and the
    final (out*recip + rb) + x + DMA after round-1 outs.
"""

import numpy as np
from contextlib import ExitStack

import ml_dtypes
import concourse.bass as bass
import concourse.bass_isa as bass_isa
import concourse.mybir as mybir
import concourse.tile as tile
from concourse import bacc
from concourse.bass_utils import run_bass_kernel_spmd

B, C, H, W = 4, 256, 64, 64
N = H * W            # 4096 tokens per image
CQ = C // 8          # 32 q/k channels
NQ = N // 2          # 2048 query tokens per core
EPS = 1e-5
P = 128
IB = 512             # i-block (psum bank of f32)
NJC = N // P         # 32 j-chunks
NSR = NJC // 2       # 16 rounds per i-block (2 j-chunks each)
NCORES = 8
NWARM = 32

f32 = mybir.dt.float32
bf16 = mybir.dt.bfloat16
FT = mybir.ActivationFunctionType
ALU = mybir.AluOpType
BF = ml_dtypes.bfloat16

_CACHE = {}


def _build():
    nc = bacc.Bacc("TRN2", target_bir_lowering=False, debug=False,
                   num_devices=NCORES)
    xbh = nc.dram_tensor("xbh", [C, N], bf16, kind="ExternalInput").ap()
    wqkT = nc.dram_tensor("wqkT", [C, 2 * CQ], bf16, kind="ExternalInput").ap()
    wvT = nc.dram_tensor("wvT", [C, C], bf16, kind="ExternalInput").ap()
    xth = nc.dram_tensor("xth", [P, NJC, C], bf16, kind="ExternalInput").ap()
    bq2 = nc.dram_tensor("bq2", [2 * CQ, 1], f32, kind="ExternalInput").ap()
    rbh = nc.dram_tensor("rbh", [P, 2], f32, kind="ExternalInput").ap()
    y = nc.dram_tensor("y", [C, NQ], bf16, kind="ExternalOutput").ap()

    with tile.TileContext(nc) as tc, ExitStack() as ctx:
        const = ctx.enter_context(tc.tile_pool(name="const", bufs=1))
        ones_col = const.tile([P, 1], bf16)
        nc.vector.memset(ones_col[:], 1.0)
        ones_row = const.tile([1, P], bf16)
        nc.vector.memset(ones_row[:], 1.0)

        wqk_sb = const.tile([P, 2, 2 * CQ], bf16)
        wq_sb = wqk_sb[:, :, 0:CQ]
        wk_sb = wqk_sb[:, :, CQ:2 * CQ]
        wv_sb = const.tile([P, 2, C], bf16)
        bq_sb = const.tile([2 * CQ, 1], f32)
        rb_sb = const.tile([P, 2], f32)

        garb = const.tile([P, 64], bf16)
        nc.vector.memset(garb[:], 0.5)

        xpool = ctx.enter_context(tc.tile_pool(name="x", bufs=1))
        xb_sb = [xpool.tile([P, N], bf16, name=f"xbsb{cc}") for cc in range(2)]

        qkv = ctx.enter_context(tc.tile_pool(name="qkv", bufs=1))
        qrep = qkv.tile([P, NQ], bf16)   # 4 row-band replicas of q
        krep = qkv.tile([P, N], bf16)    # 4 row-band replicas of k
        xt_sb = qkv.tile([P, NJC, C], bf16)  # x^T as [j-in-chunk, jc, x]

        # ---- input DMA: 4 column waves, xb0 on sync / xb1 on gpsimd /
        # weights on scalar.  Each wave overlaps the next by ONE column:
        # the WAW forces wave w+1 to start only after wave w completes, so
        # the first wave drains at full DMA bandwidth instead of sharing
        # it with the whole input, and the q-proj can start ~2us earlier.
        XW = (0, 512, 2048, 3072, N)
        nc.scalar.dma_start(out=wqk_sb[:],
                            in_=wqkT.rearrange("(k p) m -> p k m", k=2))
        for w in range(4):
            hi = min(XW[w + 1] + 1, N)
            nc.sync.dma_start(out=xb_sb[0][:, XW[w]:hi],
                              in_=xbh[0:P, XW[w]:hi])
            nc.gpsimd.dma_start(out=xb_sb[1][:, XW[w]:hi],
                                in_=xbh[P:C, XW[w]:hi])
        nc.scalar.dma_start(out=bq_sb[:], in_=bq2[:])
        nc.scalar.dma_start(out=rb_sb[:], in_=rbh[:])
        nc.scalar.dma_start(out=xt_sb[:, 0:8, :], in_=xth[:, 0:8, :])
        nc.gpsimd.dma_start(out=xt_sb[:, 8:20, :], in_=xth[:, 8:20, :])
        nc.scalar.dma_start(out=xt_sb[:, 20:NJC, :], in_=xth[:, 20:NJC, :])
        # wv is only needed by the per-block Wv@Z finish (round 18+)
        nc.scalar.dma_start(out=wv_sb[:],
                            in_=wvT.rearrange("(k p) m -> p k m", k=2))

        # PE warmup during the input-DMA window: dependency-free tiny
        # matmuls keep the PE busy past the HAM activity window so the
        # projection phase starts at the full clock.
        with tc.tile_pool(name="warm_ps", bufs=1, space="PSUM") as wps:
            wtile = wps.tile([1, 64], f32, name="warm")
            for _ in range(NWARM):
                nc.tensor.matmul(wtile[:], lhsT=ones_col[:], rhs=garb[:],
                                 start=True, stop=True)

        # ---- projections, emitted in column-availability order so the
        # PE is never waiting on a later DMA wave while earlier-column
        # work exists ----
        def emit_q(nb, pool, tag):
            # single col-group: replicas to the other PE row bands come
            # from copies, not extra matmul streams.  Bias is applied by
            # the DVE eviction so the ACT queue stays exp-only.
            ps = pool.tile([CQ, IB], f32, name="qps", tag=tag)
            for k in range(2):
                nc.tensor.matmul(
                    ps[:],
                    lhsT=wq_sb[:, k, :],
                    rhs=xb_sb[k][:, nb * IB:(nb + 1) * IB],
                    start=(k == 0), stop=(k == 1))
            nc.vector.tensor_scalar_add(
                qrep[0:CQ, nb * IB:(nb + 1) * IB], ps[:], bq_sb[0:CQ, :])
            nc.vector.tensor_copy(
                out=qrep[CQ:2 * CQ, nb * IB:(nb + 1) * IB],
                in_=qrep[0:CQ, nb * IB:(nb + 1) * IB])

        def emit_k(nb, pool, tag):
            ps = pool.tile([CQ, IB], f32, name="kps", tag=tag)
            for k in range(2):
                nc.tensor.matmul(
                    ps[:],
                    lhsT=wk_sb[:, k, :],
                    rhs=xb_sb[k][:, nb * IB:(nb + 1) * IB],
                    start=(k == 0), stop=(k == 1))
            nc.vector.tensor_copy(
                out=krep[0:CQ, nb * IB:(nb + 1) * IB], in_=ps[:])
            nc.vector.tensor_copy(
                out=krep[CQ:2 * CQ, nb * IB:(nb + 1) * IB],
                in_=krep[0:CQ, nb * IB:(nb + 1) * IB])

        with tc.tile_pool(name="proj_ps", bufs=4, space="PSUM") as pps, \
                tc.tile_pool(name="pad_ps", bufs=1, space="PSUM") as pad_ps:
            padt = pad_ps.tile([1, 64], f32, name="padt")

            def pad(n):
                # dependency-free filler matmuls: keep the PE continuously
                # busy across DMA-wave seams so the pstate never drops back
                # to the slow ramp
                for _ in range(n):
                    nc.tensor.matmul(padt[:], lhsT=ones_col[:], rhs=garb[:],
                                     start=True, stop=True)

            emit_q(0, pps, "pps")
            emit_k(0, pps, "pps")
            emit_k(1, pps, "pps")
            pad(16)

        # ---- attention main loop ----
        sp_ps = ctx.enter_context(tc.tile_pool(name="sp_ps", bufs=3, space="PSUM"))
        out_ps = ctx.enter_context(tc.tile_pool(name="out_ps", bufs=1, space="PSUM"))
        pp_pool = ctx.enter_context(tc.tile_pool(name="ppp", bufs=4))
        acc_pool = ctx.enter_context(tc.tile_pool(name="accp", bufs=4))
        osb_pool = ctx.enter_context(tc.tile_pool(name="osbp", bufs=2))
        ysb_pool = ctx.enter_context(tc.tile_pool(name="ysbp", bufs=4))
        rec_pool = ctx.enter_context(tc.tile_pool(name="recp", bufs=2))
        bc_pool = ctx.enter_context(tc.tile_pool(name="bcp", bufs=2))

        NIB = NQ // IB
        NSRT = NIB * NSR   # total rounds

        def emit_s(sr):
            """S^T matmul pair for round sr: two concurrent 32-row-band MMs.

            Even rounds use PE row bands 0/32, odd rounds 64/96, so the two
            adjacently emitted pairs stream 4-way concurrently.
            """
            ib, lsr = divmod(sr, NSR)
            i0 = ib * IB
            if sr < 18:       # band-2/3 replica DMAs still in flight
                bands = (0, CQ)
            else:             # all four bands live; alternate pairs
                base = (sr % 2) * 2 * CQ
                bands = (base, base + CQ)
            sp = sp_ps.tile([P, 2, IB], f32, name="sp")
            for g in range(2):
                jc = 2 * lsr + g
                bb = bands[g]
                nc.tensor.matmul(
                    sp[:, g, :],
                    lhsT=krep[bb:bb + CQ, jc * P:(jc + 1) * P],
                    rhs=qrep[bb:bb + CQ, i0:i0 + IB],
                    start=True, stop=True,
                    tile_position=(bb, 0))
            return sp

        def stage_a(st):
            """Evict the Z psum, reduce both rowsum chains, reciprocal,
            broadcast.  The rs psum tile is allocated here and freed ~1
            round later, so it never pinches the sp slot rotation."""
            zsb = osb_pool.tile([P, 2, IB], bf16, name="zsb")
            nc.vector.tensor_copy(out=zsb[:], in_=st["outp"][:])
            st["zsb"] = zsb
            epi = sp_ps.tile([P, 2, IB], f32, name="sp")
            rs = epi[0:1, 0, :]
            for g in range(2):
                nc.tensor.matmul(rs, lhsT=ones_col[:], rhs=st["accA"][:, g, :],
                                 start=(g == 0), stop=False)
            for g in range(2):
                nc.tensor.matmul(rs, lhsT=ones_col[:], rhs=st["accB"][:, g, :],
                                 start=False, stop=(g == 1))
            recip = rec_pool.tile([1, IB], f32, name="recip")
            nc.vector.reciprocal_approx_fast(out=recip[:], in_=rs)
            recb = rec_pool.tile([1, IB], bf16, name="recb")
            nc.vector.tensor_copy(out=recb[:], in_=recip[:])
            if st["last"]:
                # tail: PE rank-1 broadcast (lowest latency; Z psum is free)
                bc = st["outp"][:, 0, :]
                nc.tensor.matmul(bc, lhsT=ones_row[:], rhs=recb[:],
                                 start=True, stop=True)
                bcs = bc_pool.tile([P, IB], bf16, name="bcs")
                nc.vector.tensor_copy(out=bcs[:], in_=bc)
            else:
                bcs = bc_pool.tile([P, IB], bf16, name="bcs")
                nc.gpsimd.partition_broadcast(out_ap=bcs[:], in_ap=recb[:],
                                              channels=P)
            st["bcs"] = bcs

        def stage_b(st):
            """out = Wv @ Z: 4 matmuls into an sp psum slot."""
            wvout = sp_ps.tile([P, 2, IB], f32, name="sp")
            zsb = st["zsb"]
            for cc in range(2):
                for k in range(2):
                    nc.tensor.matmul(
                        wvout[:, cc, :],
                        lhsT=wv_sb[:, k, cc * P:(cc + 1) * P],
                        rhs=zsb[:, k, :],
                        start=(k == 0), stop=(k == 1))
            st["wvout"] = wvout

        def stage_c(st):
            ob = osb_pool.tile([P, 2, IB], bf16, name="ob")
            nc.vector.tensor_copy(out=ob[:], in_=st["wvout"][:])
            st["ob"] = ob

        def _store_half(st, cc):
            """Normalize + residual + output DMA for one channel half,
            all bf16 (DVE 2x mode); split across two rounds so the DVE
            burst never backs up the exp stream's acc adds."""
            bcs, ob, i0 = st["bcs"], st["ob"], st["i0"]
            tmp = ysb_pool.tile([P, IB], bf16, name="tmp")
            nc.vector.tensor_mul(tmp[:], ob[:, cc, :], bcs[:])
            ysb = ysb_pool.tile([P, IB], bf16, name="ysb")
            nc.vector.scalar_tensor_tensor(
                out=ysb[:], in0=tmp[:], scalar=rb_sb[:, cc:cc + 1],
                in1=xb_sb[cc][:, i0:i0 + IB],
                op0=ALU.add, op1=ALU.add)
            nc.sync.dma_start(out=y[cc * P:(cc + 1) * P, i0:i0 + IB],
                              in_=ysb[:])

        def stage_d0(st):
            _store_half(st, 0)

        def stage_d1(st):
            _store_half(st, 1)

        def emit_z(ppt, zlsr, zoutp):
            """Z += x_chunk @ P^T for round zlsr (runs one round late, so
            the previous block's finish fits before this block's Z starts)."""
            for g in range(2):
                jc = 2 * zlsr + g
                for kc in range(2):
                    nc.tensor.matmul(
                        zoutp[:, kc, :],
                        lhsT=xt_sb[:, jc, kc * P:(kc + 1) * P],
                        rhs=ppt[:, g, :],
                        start=(zlsr == 0 and g == 0),
                        stop=(zlsr == NSR - 1 and g == 1))

        # software pipeline: S pairs prefetch 2-3 rounds ahead; Z matmuls
        # lag one round behind their exp.
        sps = {0: emit_s(0), 1: emit_s(1)}
        outp = None
        accs = None
        prev_pp = None
        pending = []
        for sr in range(NSRT):
            ib, lsr = divmod(sr, NSR)
            i0 = ib * IB
            if lsr == 0:
                outp = out_ps.tile([P, 2, IB], f32, name="outp")
                accs = [None, None]
            ch = lsr // (NSR // 2)   # rowsum chain A: rounds 0-7, B: 8-15
            if lsr % (NSR // 2) == 0:
                # chain start: exp writes the accumulator tile directly
                ppt = acc_pool.tile([P, 2, IB], bf16, name=f"acc{ch}")
                accs[ch] = ppt
                nc.scalar.activation(out=ppt[:], in_=sps.pop(sr)[:],
                                     func=FT.Exp)
                add_after = None
            else:
                ppt = pp_pool.tile([P, 2, IB], bf16, name="pp")
                nc.scalar.activation(out=ppt[:], in_=sps.pop(sr)[:],
                                     func=FT.Exp)
                add_after = accs[ch]
            # k/q projections are fused into the main loop so the
            # in-order PE never idles waiting for late DMA waves: chunk
            # consumption tracks the chained input waves.
            if sr % 2 == 1 and 2 <= (sr + 3) // 2 <= 7:
                emit_k((sr + 3) // 2, sp_ps, "sp")
            if sr in (10, 12, 14):
                emit_q(sr // 2 - 4, sp_ps, "sp")
            if sr == 4:    # k bands 0/1 -> 2/3, first half (needed sr>=18)
                nc.sync.dma_start(out=krep[2 * CQ:P, 0:2048],
                                  in_=krep[0:2 * CQ, 0:2048])
            if sr == 12:   # second half (needed sr>=26)
                nc.sync.dma_start(out=krep[2 * CQ:P, 2048:N],
                                  in_=krep[0:2 * CQ, 2048:N])
            if sr == 15:   # q bands 0/1 -> 2/3 (needed sr>=18)
                nc.gpsimd.dma_start(out=qrep[2 * CQ:P, :],
                                    in_=qrep[0:2 * CQ, :])
            if lsr % 2 == 0:
                for dd in (2, 3):
                    if sr + dd < NSRT:
                        sps[sr + dd] = emit_s(sr + dd)
            # lagged Z matmuls must read the previous ppt BEFORE this
            # round's in-place chain add mutates it (chain-start rounds
            # alias ppt and the accumulator)
            if prev_pp is not None:
                emit_z(*prev_pp)
            if add_after is not None:
                nc.vector.tensor_add(add_after[:], add_after[:], ppt[:])
            prev_pp = (ppt, lsr, outp)
            # previous block's deferred epilogue stages
            if pending and lsr == pending[0][0]:
                _, st, fn = pending.pop(0)
                fn(st)
            if lsr < NSR - 1:
                continue
            st = {"outp": outp, "accA": accs[0], "accB": accs[1], "i0": i0,
                  "last": sr == NSRT - 1}
            pending = [(0, st, stage_a), (2, st, stage_b),
                       (3, st, stage_c), (6, st, stage_d0),
                       (9, st, stage_d1)]
        emit_z(*prev_pp)   # final round's Z matmuls
        for _, st, fn in pending:   # last i-block epilogue
            fn(st)

    nc.compile()
    return nc


def _get_nc():
    if "nc" not in _CACHE:
        _CACHE["nc"] = _build()
    return _CACHE["nc"]


def _fold_bn(w, b, g, beta, mean, var):
    s = g / np.sqrt(var + EPS)
    return w * s[:, None], b * s + beta - mean * s


def _in_maps(inputs):
    gx = np.asarray(inputs["x"], np.float32)
    gamma = float(np.asarray(inputs["gamma"]).reshape(-1)[0])
    wq, bq_ = _fold_bn(*[np.asarray(inputs[k], np.float32) for k in
                         ("q_w", "q_b", "q_g", "q_beta", "q_mean", "q_var")])
    wk, _bk = _fold_bn(*[np.asarray(inputs[k], np.float32) for k in
                         ("k_w", "k_b", "k_g", "k_beta", "k_mean", "k_var")])
    wv, bv_ = _fold_bn(*[np.asarray(inputs[k], np.float32) for k in
                         ("v_w", "v_b", "v_g", "v_beta", "v_mean", "v_var")])
    wqkT = np.ascontiguousarray(
        np.concatenate([wq.T, wk.T], axis=1).astype(BF))
    wvT = np.ascontiguousarray((gamma * wv).T.astype(BF))
    bq2 = np.ascontiguousarray(np.tile(bq_.reshape(CQ, 1), (2, 1)))
    rbh = np.ascontiguousarray((gamma * bv_).reshape(2, P).T)
    maps = []
    for core in range(NCORES):
        b, h = divmod(core, 2)
        xf = gx[b].reshape(C, N).astype(BF)
        if h == 1:  # own query-half columns first; key order is irrelevant
            xf = np.concatenate([xf[:, NQ:], xf[:, :NQ]], axis=1)
        maps.append({
            "xbh": np.ascontiguousarray(xf),
            "xth": np.ascontiguousarray(
                np.asarray(xf.T).reshape(NJC, P, C).transpose(1, 0, 2)),
            "wqkT": wqkT, "wvT": wvT,
            "bq2": bq2, "rbh": rbh,
        })
    return maps


def _gather(results):
    out = np.empty((B, C, N), np.float32)
    for core in range(NCORES):
        b, h = divmod(core, 2)
        out[b][:, h * NQ:(h + 1) * NQ] = np.asarray(
            results[core]["y"]).astype(np.float32)
    return out.reshape(B, C, H, W)


def _run(inputs, **kw):
    nc = _get_nc()
    res = run_bass_kernel_spmd(nc, _in_maps(inputs),
                               core_ids=list(range(NCORES)), **kw)
    return res


def kernel(**inputs) -> np.ndarray:
    return _gather(_run(inputs).results)


# revision 26
# speedup vs baseline: 1.0729x; 1.0729x over previous
"""Trainium2 Bass kernel for nn_AttentionLayer (self-attention over 64x64 images).

Computation (per batch image b):
    xf = x[b].reshape(C, N)                     # C=256, N=4096
    q = BN(Wq @ xf + bq)   -> [32, N]
    k = BN(Wk @ xf + bk)   -> [32, N]
    v = BN(Wv @ xf + bv)   -> [256, N]
    attn = softmax_j(q^T k) -> [N, N]
    out = v @ attn^T        -> [256, N]
    y = gamma * out + xf

Sharding: 8 cores = 4 batches x 2 query-row halves; no collectives.

Host-side algebra folds (all exact):
  - BN folded into weights/bias.
  - k bias drops entirely (adds a per-query constant to every logit row ->
    cancels in softmax); q bias kept (its logit term varies over keys).
  - v bias passes through softmax unchanged (rows sum to 1), so
    y = (gamma*vhat @ P)*recip + (x + gamma*bv); gamma folds into Wv and the
    residual bias rb = gamma*bv is added per-channel in the epilogue.
  - Each core's xbh has its own query-half columns FIRST (keys are
    order-invariant), so the query slice is a view of the key tensor and the
    bf16 x tile doubles as the residual.

Device (per core, all matmuls bf16 in / f32 psum):
  - Input DMAs in 3 column waves issued from three different queues
    (sync/gpsimd/scalar) so the ~0.7us per-issue cost is paid in parallel
    and the first projection matmul starts ~9.5us in.
  - q/k proj: 2 col-group replica matmuls -> [64, N*] (col-group pairs
    stream near-concurrently on the PE); q evicted via ACT Identity+bias,
    k via DVE copy.  Bands 64-127 are then filled by one SBUF->SBUF DMA
    per tensor, so S rounds can use 4 distinct PE row bands.
  - V^T[j, c] = x^T Wv^T computed directly in [j, c] layout.
  - Main loop over 4 i-blocks x 16 rounds of 2 key-chunks: S^T row-band
    matmul pair -> exp on ScalarE (no max subtraction; |S|<60 fits f32/bf16)
    -> P^T bf16 -> out psum accumulation + bf16 rowsum chains on DVE.
    Consecutive rounds use different PE row-band pairs (0/32 vs 64/96), so
    the adjacently emitted S pairs of rounds sr+2 and sr+3 stream 4-way
    concurrently.
  - Epilogue per i-block is split so no PE instruction ever waits on the
    rowsum->reciprocal chain: at the block's last round only the psum
    eviction happens; the rowsum matmuls + reciprocal + GpSimd
    partition_broadcast flush after the NEXT block's round-0 outs, and the
    final (out*recip + rb) + x + DMA after round-1 outs.
"""

import numpy as np
from contextlib import ExitStack

import ml_dtypes
import concourse.bass as bass
import concourse.bass_isa as bass_isa
import concourse.mybir as mybir
import concourse.tile as tile
from concourse import bacc
from concourse.bass_utils import run_bass_kernel_spmd

B, C, H, W = 4, 256, 64, 64
N = H * W            # 4096 tokens per image
CQ = C // 8          # 32 q/k channels
NQ = N // 2          # 2048 query tokens per core
EPS = 1e-5
P = 128
IB = 512             # i-block (psum bank of f32)
NJC = N // P         # 32 j-chunks
NSR = NJC // 2       # 16 rounds per i-block (2 j-chunks each)
NCORES = 8
NWARM = 32

f32 = mybir.dt.float32
bf16 = mybir.dt.bfloat16
FT = mybir.ActivationFunctionType
ALU = mybir.AluOpType
BF = ml_dtypes.bfloat16

_CACHE = {}


def _build():
    nc = bacc.Bacc("TRN2", target_bir_lowering=False, debug=False,
                   num_devices=NCORES)
    xbh = nc.dram_tensor("xbh", [C, N], bf16, kind="ExternalInput").ap()
    wqkT = nc.dram_tensor("wqkT", [C, 2 * CQ], bf16, kind="ExternalInput").ap()
    wvT = nc.dram_tensor("wvT", [C, C], bf16, kind="ExternalInput").ap()
    xth = nc.dram_tensor("xth", [P, NJC, C], bf16, kind="ExternalInput").ap()
    bq2 = nc.dram_tensor("bq2", [2 * CQ, 1], f32, kind="ExternalInput").ap()
    rbh = nc.dram_tensor("rbh", [P, 2], f32, kind="ExternalInput").ap()
    y = nc.dram_tensor("y", [C, NQ], bf16, kind="ExternalOutput").ap()

    with tile.TileContext(nc) as tc, ExitStack() as ctx:
        const = ctx.enter_context(tc.tile_pool(name="const", bufs=1))
        ones_col = const.tile([P, 1], bf16)
        nc.vector.memset(ones_col[:], 1.0)
        ones_row = const.tile([1, P], bf16)
        nc.vector.memset(ones_row[:], 1.0)

        wqk_sb = const.tile([P, 2, 2 * CQ], bf16)
        wq_sb = wqk_sb[:, :, 0:CQ]
        wk_sb = wqk_sb[:, :, CQ:2 * CQ]
        wv_sb = const.tile([P, 2, C], bf16)
        bq_sb = const.tile([2 * CQ, 1], f32)
        rb_sb = const.tile([P, 2], f32)

        garb = const.tile([P, 64], bf16)
        nc.vector.memset(garb[:], 0.5)

        xpool = ctx.enter_context(tc.tile_pool(name="x", bufs=1))
        xb_sb = [xpool.tile([P, N], bf16, name=f"xbsb{cc}") for cc in range(2)]

        qkv = ctx.enter_context(tc.tile_pool(name="qkv", bufs=1))
        qrep = qkv.tile([P, NQ], bf16)   # 4 row-band replicas of q
        krep = qkv.tile([P, N], bf16)    # 4 row-band replicas of k
        xt_sb = qkv.tile([P, NJC, C], bf16)  # x^T as [j-in-chunk, jc, x]

        # ---- input DMA: 4 column waves, xb0 on sync / xb1 on gpsimd /
        # weights on scalar.  Each wave overlaps the next by ONE column:
        # the WAW forces wave w+1 to start only after wave w completes, so
        # the first wave drains at full DMA bandwidth instead of sharing
        # it with the whole input, and the q-proj can start ~2us earlier.
        XW = (0, 512, 1536, 2560, 3584, N)
        nc.scalar.dma_start(out=wqk_sb[:],
                            in_=wqkT.rearrange("(k p) m -> p k m", k=2))
        for w in range(5):
            hi = min(XW[w + 1] + 1, N)
            nc.sync.dma_start(out=xb_sb[0][:, XW[w]:hi],
                              in_=xbh[0:P, XW[w]:hi])
            nc.gpsimd.dma_start(out=xb_sb[1][:, XW[w]:hi],
                                in_=xbh[P:C, XW[w]:hi])
        nc.scalar.dma_start(out=bq_sb[:], in_=bq2[:])
        nc.scalar.dma_start(out=rb_sb[:], in_=rbh[:])
        # xt waves are chained by a 1-chunk WAW overlap so their payloads
        # never compete with the earlier xbh waves for DMA bandwidth
        for j0, j1 in ((0, 6), (5, 14), (13, 24), (23, NJC)):
            nc.scalar.dma_start(out=xt_sb[:, j0:j1, :], in_=xth[:, j0:j1, :])
        # wv is only needed by the per-block Wv@Z finish (round 18+)
        nc.scalar.dma_start(out=wv_sb[:],
                            in_=wvT.rearrange("(k p) m -> p k m", k=2))

        # PE warmup during the input-DMA window: dependency-free tiny
        # matmuls keep the PE busy past the HAM activity window so the
        # projection phase starts at the full clock.
        with tc.tile_pool(name="warm_ps", bufs=1, space="PSUM") as wps:
            wtile = wps.tile([1, 64], f32, name="warm")
            for _ in range(NWARM):
                nc.tensor.matmul(wtile[:], lhsT=ones_col[:], rhs=garb[:],
                                 start=True, stop=True)

        # ---- projections, emitted in column-availability order so the
        # PE is never waiting on a later DMA wave while earlier-column
        # work exists ----
        def emit_q(nb, pool, tag):
            # single col-group: replicas to the other PE row bands come
            # from copies, not extra matmul streams.  Bias is applied by
            # the DVE eviction so the ACT queue stays exp-only.
            ps = pool.tile([CQ, IB], f32, name="qps", tag=tag)
            for k in range(2):
                nc.tensor.matmul(
                    ps[:],
                    lhsT=wq_sb[:, k, :],
                    rhs=xb_sb[k][:, nb * IB:(nb + 1) * IB],
                    start=(k == 0), stop=(k == 1))
            nc.vector.tensor_scalar_add(
                qrep[0:CQ, nb * IB:(nb + 1) * IB], ps[:], bq_sb[0:CQ, :])
            nc.vector.tensor_copy(
                out=qrep[CQ:2 * CQ, nb * IB:(nb + 1) * IB],
                in_=qrep[0:CQ, nb * IB:(nb + 1) * IB])

        def emit_k(nb, pool, tag):
            ps = pool.tile([CQ, IB], f32, name="kps", tag=tag)
            for k in range(2):
                nc.tensor.matmul(
                    ps[:],
                    lhsT=wk_sb[:, k, :],
                    rhs=xb_sb[k][:, nb * IB:(nb + 1) * IB],
                    start=(k == 0), stop=(k == 1))
            nc.vector.tensor_copy(
                out=krep[0:CQ, nb * IB:(nb + 1) * IB], in_=ps[:])
            nc.vector.tensor_copy(
                out=krep[CQ:2 * CQ, nb * IB:(nb + 1) * IB],
                in_=krep[0:CQ, nb * IB:(nb + 1) * IB])

        with tc.tile_pool(name="proj_ps", bufs=4, space="PSUM") as pps, \
                tc.tile_pool(name="pad_ps", bufs=1, space="PSUM") as pad_ps:
            padt = pad_ps.tile([1, 64], f32, name="padt")

            def pad(n):
                # dependency-free filler matmuls: keep the PE continuously
                # busy across DMA-wave seams so the pstate never drops back
                # to the slow ramp
                for _ in range(n):
                    nc.tensor.matmul(padt[:], lhsT=ones_col[:], rhs=garb[:],
                                     start=True, stop=True)

            emit_q(0, pps, "pps")
            emit_k(0, pps, "pps")
            emit_k(1, pps, "pps")
            pad(16)

        # ---- attention main loop ----
        sp_ps = ctx.enter_context(tc.tile_pool(name="sp_ps", bufs=3, space="PSUM"))
        out_ps = ctx.enter_context(tc.tile_pool(name="out_ps", bufs=1, space="PSUM"))
        pp_pool = ctx.enter_context(tc.tile_pool(name="ppp", bufs=4))
        acc_pool = ctx.enter_context(tc.tile_pool(name="accp", bufs=4))
        osb_pool = ctx.enter_context(tc.tile_pool(name="osbp", bufs=2))
        ysb_pool = ctx.enter_context(tc.tile_pool(name="ysbp", bufs=4))
        rec_pool = ctx.enter_context(tc.tile_pool(name="recp", bufs=2))
        bc_pool = ctx.enter_context(tc.tile_pool(name="bcp", bufs=2))

        NIB = NQ // IB
        NSRT = NIB * NSR   # total rounds

        def emit_s(sr):
            """S^T matmul pair for round sr: two concurrent 32-row-band MMs.

            Even rounds use PE row bands 0/32, odd rounds 64/96, so the two
            adjacently emitted pairs stream 4-way concurrently.
            """
            ib, lsr = divmod(sr, NSR)
            i0 = ib * IB
            if sr < 18:       # band-2/3 replica DMAs still in flight
                bands = (0, CQ)
            else:             # all four bands live; alternate pairs
                base = (sr % 2) * 2 * CQ
                bands = (base, base + CQ)
            sp = sp_ps.tile([P, 2, IB], f32, name="sp")
            for g in range(2):
                jc = 2 * lsr + g
                bb = bands[g]
                nc.tensor.matmul(
                    sp[:, g, :],
                    lhsT=krep[bb:bb + CQ, jc * P:(jc + 1) * P],
                    rhs=qrep[bb:bb + CQ, i0:i0 + IB],
                    start=True, stop=True,
                    tile_position=(bb, 0))
            return sp

        def stage_a(st):
            """Evict the Z psum, reduce both rowsum chains, reciprocal,
            broadcast.  The rs psum tile is allocated here and freed ~1
            round later, so it never pinches the sp slot rotation."""
            zsb = osb_pool.tile([P, 2, IB], bf16, name="zsb")
            nc.vector.tensor_copy(out=zsb[:], in_=st["outp"][:])
            st["zsb"] = zsb
            epi = sp_ps.tile([P, 2, IB], f32, name="sp")
            rs = epi[0:1, 0, :]
            for g in range(2):
                nc.tensor.matmul(rs, lhsT=ones_col[:], rhs=st["accA"][:, g, :],
                                 start=(g == 0), stop=False)
            for g in range(2):
                nc.tensor.matmul(rs, lhsT=ones_col[:], rhs=st["accB"][:, g, :],
                                 start=False, stop=(g == 1))
            recip = rec_pool.tile([1, IB], f32, name="recip")
            nc.vector.reciprocal_approx_fast(out=recip[:], in_=rs)
            recb = rec_pool.tile([1, IB], bf16, name="recb")
            nc.vector.tensor_copy(out=recb[:], in_=recip[:])
            if st["last"]:
                # tail: PE rank-1 broadcast (lowest latency; Z psum is free)
                bc = st["outp"][:, 0, :]
                nc.tensor.matmul(bc, lhsT=ones_row[:], rhs=recb[:],
                                 start=True, stop=True)
                bcs = bc_pool.tile([P, IB], bf16, name="bcs")
                nc.vector.tensor_copy(out=bcs[:], in_=bc)
            else:
                bcs = bc_pool.tile([P, IB], bf16, name="bcs")
                nc.gpsimd.partition_broadcast(out_ap=bcs[:], in_ap=recb[:],
                                              channels=P)
            st["bcs"] = bcs

        def stage_b(st):
            """out = Wv @ Z: 4 matmuls into an sp psum slot."""
            wvout = sp_ps.tile([P, 2, IB], f32, name="sp")
            zsb = st["zsb"]
            for cc in range(2):
                for k in range(2):
                    nc.tensor.matmul(
                        wvout[:, cc, :],
                        lhsT=wv_sb[:, k, cc * P:(cc + 1) * P],
                        rhs=zsb[:, k, :],
                        start=(k == 0), stop=(k == 1))
            st["wvout"] = wvout

        def stage_c(st):
            ob = osb_pool.tile([P, 2, IB], bf16, name="ob")
            nc.vector.tensor_copy(out=ob[:], in_=st["wvout"][:])
            st["ob"] = ob

        def _store_half(st, cc):
            """Normalize + residual + output DMA for one channel half,
            all bf16 (DVE 2x mode); split across two rounds so the DVE
            burst never backs up the exp stream's acc adds."""
            bcs, ob, i0 = st["bcs"], st["ob"], st["i0"]
            tmp = ysb_pool.tile([P, IB], bf16, name="tmp")
            nc.vector.tensor_mul(tmp[:], ob[:, cc, :], bcs[:])
            ysb = ysb_pool.tile([P, IB], bf16, name="ysb")
            nc.vector.scalar_tensor_tensor(
                out=ysb[:], in0=tmp[:], scalar=rb_sb[:, cc:cc + 1],
                in1=xb_sb[cc][:, i0:i0 + IB],
                op0=ALU.add, op1=ALU.add)
            nc.sync.dma_start(out=y[cc * P:(cc + 1) * P, i0:i0 + IB],
                              in_=ysb[:])

        def stage_d0(st):
            _store_half(st, 0)

        def stage_d1(st):
            _store_half(st, 1)

        def emit_z(ppt, zlsr, zoutp):
            """Z += x_chunk @ P^T for round zlsr (runs one round late, so
            the previous block's finish fits before this block's Z starts)."""
            for g in range(2):
                jc = 2 * zlsr + g
                for kc in range(2):
                    nc.tensor.matmul(
                        zoutp[:, kc, :],
                        lhsT=xt_sb[:, jc, kc * P:(kc + 1) * P],
                        rhs=ppt[:, g, :],
                        start=(zlsr == 0 and g == 0),
                        stop=(zlsr == NSR - 1 and g == 1))

        # software pipeline: S pairs prefetch 2-3 rounds ahead; Z matmuls
        # lag one round behind their exp.
        sps = {0: emit_s(0), 1: emit_s(1)}
        outp = None
        accs = None
        prev_pp = None
        pending = []
        for sr in range(NSRT):
            ib, lsr = divmod(sr, NSR)
            i0 = ib * IB
            if lsr == 0:
                outp = out_ps.tile([P, 2, IB], f32, name="outp")
                accs = [None, None]
            ch = lsr // (NSR // 2)   # rowsum chain A: rounds 0-7, B: 8-15
            if lsr % (NSR // 2) == 0:
                # chain start: exp writes the accumulator tile directly
                ppt = acc_pool.tile([P, 2, IB], bf16, name=f"acc{ch}")
                accs[ch] = ppt
                nc.scalar.activation(out=ppt[:], in_=sps.pop(sr)[:],
                                     func=FT.Exp)
                add_after = None
            else:
                ppt = pp_pool.tile([P, 2, IB], bf16, name="pp")
                nc.scalar.activation(out=ppt[:], in_=sps.pop(sr)[:],
                                     func=FT.Exp)
                add_after = accs[ch]
            # k/q projections are fused into the main loop so the
            # in-order PE never idles waiting for late DMA waves: chunk
            # consumption tracks the chained input waves.
            if sr % 2 == 1 and 2 <= (sr + 3) // 2 <= 7:
                emit_k((sr + 3) // 2, sp_ps, "sp")
            if sr in (10, 12, 14):
                emit_q(sr // 2 - 4, sp_ps, "sp")
            if sr == 4:    # k bands 0/1 -> 2/3, first half (needed sr>=18)
                nc.sync.dma_start(out=krep[2 * CQ:P, 0:2048],
                                  in_=krep[0:2 * CQ, 0:2048])
            if sr == 12:   # second half (needed sr>=26)
                nc.sync.dma_start(out=krep[2 * CQ:P, 2048:N],
                                  in_=krep[0:2 * CQ, 2048:N])
            if sr == 15:   # q bands 0/1 -> 2/3 (needed sr>=18)
                nc.gpsimd.dma_start(out=qrep[2 * CQ:P, :],
                                    in_=qrep[0:2 * CQ, :])
            if lsr % 2 == 0:
                for dd in (2, 3):
                    if sr + dd < NSRT:
                        sps[sr + dd] = emit_s(sr + dd)
            # lagged Z matmuls must read the previous ppt BEFORE this
            # round's in-place chain add mutates it (chain-start rounds
            # alias ppt and the accumulator)
            if prev_pp is not None:
                emit_z(*prev_pp)
            if add_after is not None:
                nc.vector.tensor_add(add_after[:], add_after[:], ppt[:])
            prev_pp = (ppt, lsr, outp)
            # previous block's deferred epilogue stages
            if pending and lsr == pending[0][0]:
                _, st, fn = pending.pop(0)
                fn(st)
            if lsr < NSR - 1:
                continue
            st = {"outp": outp, "accA": accs[0], "accB": accs[1], "i0": i0,
                  "last": sr == NSRT - 1}
            pending = [(0, st, stage_a), (2, st, stage_b),
                       (3, st, stage_c), (6, st, stage_d0),
                       (9, st, stage_d1)]
        emit_z(*prev_pp)   # final round's Z matmuls
        for _, st, fn in pending:   # last i-block epilogue
            fn(st)

    nc.compile()
    return nc


def _get_nc():
    if "nc" not in _CACHE:
        _CACHE["nc"] = _build()
    return _CACHE["nc"]


def _fold_bn(w, b, g, beta, mean, var):
    s = g / np.sqrt(var + EPS)
    return w * s[:, None], b * s + beta - mean * s


def _in_maps(inputs):
    gx = np.asarray(inputs["x"], np.float32)
    gamma = float(np.asarray(inputs["gamma"]).reshape(-1)[0])
    wq, bq_ = _fold_bn(*[np.asarray(inputs[k], np.float32) for k in
                         ("q_w", "q_b", "q_g", "q_beta", "q_mean", "q_var")])
    wk, _bk = _fold_bn(*[np.asarray(inputs[k], np.float32) for k in
                         ("k_w", "k_b", "k_g", "k_beta", "k_mean", "k_var")])
    wv, bv_ = _fold_bn(*[np.asarray(inputs[k], np.float32) for k in
                         ("v_w", "v_b", "v_g", "v_beta", "v_mean", "v_var")])
    wqkT = np.ascontiguousarray(
        np.concatenate([wq.T, wk.T], axis=1).astype(BF))
    wvT = np.ascontiguousarray((gamma * wv).T.astype(BF))
    bq2 = np.ascontiguousarray(np.tile(bq_.reshape(CQ, 1), (2, 1)))
    rbh = np.ascontiguousarray((gamma * bv_).reshape(2, P).T)
    maps = []
    for core in range(NCORES):
        b, h = divmod(core, 2)
        xf = gx[b].reshape(C, N).astype(BF)
        if h == 1:  # own query-half columns first; key order is irrelevant
            xf = np.concatenate([xf[:, NQ:], xf[:, :NQ]], axis=1)
        maps.append({
            "xbh": np.ascontiguousarray(xf),
            "xth": np.ascontiguousarray(
                np.asarray(xf.T).reshape(NJC, P, C).transpose(1, 0, 2)),
            "wqkT": wqkT, "wvT": wvT,
            "bq2": bq2, "rbh": rbh,
        })
    return maps


def _gather(results):
    out = np.empty((B, C, N), np.float32)
    for core in range(NCORES):
        b, h = divmod(core, 2)
        out[b][:, h * NQ:(h + 1) * NQ] = np.asarray(
            results[core]["y"]).astype(np.float32)
    return out.reshape(B, C, H, W)


def _run(inputs, **kw):
    nc = _get_nc()
    res = run_bass_kernel_spmd(nc, _in_maps(inputs),
                               core_ids=list(range(NCORES)), **kw)
    return res


def kernel(**inputs) -> np.ndarray:
    return _gather(_run(inputs).results)
